# revision 16
# baseline (speedup 1.0000x reference)
"""Trainium2 Bass kernel for LocalSpatialSimilarity.

Per sample (B=16, C=256, H=W=64, N=4096 pixels):
  s[p]  = sum_c x[c,p]                (channel sum)
  q[p]  = sum_c x[c,p]^2              (channel sum of squares)
  box   = 3x3 zero-padded box-sum of s (reshaped to 64x64)
  sim   = (box/9 * s) / sqrt(max(q * box^2 * 256/81, 1e-12))
  out   = softmax over p of (mask ? -inf : -sim)
        = (mask ? 0 : exp(-sim)) / total        (sim bounded in [-1,1] -> no
                                                 max-subtraction needed)

Sharding: pure data parallel, 2 samples per core across 8 cores.

v4 design (evolved through perfetto-trace iterations):
  * x streams in eight 1 MiB half-chunks over BOTH HWDGE rings (channel
    chunk 0 on sync, chunk 1 on scalar) -- measured ~430 GB/s aggregate.
    SWDGE proved ~150 GB/s, so it only carries the tiny reshape/out DMAs
    (where it avoids FIFO-queueing behind the big loads).
  * Channel reductions on the PE: fp32r for sum(x) (DMA writes through
    f32r-typed APs so the BIR verifier sees rounded producers), bf16 for
    sum(x^2).  Sliding 8-wide indicator band stationaries land pixel
    block g on psum partition g; s-matmuls and q-matmuls are emitted in
    separate bursts so a pending square never blocks ready s-matmuls in
    the PE FIFO.
  * Squares split between ACT (chunk 0) and DVE (chunk 1) so neither
    queue stalls, and so DMA-issue slots never sit behind a 2us square.
  * ~5us of dummy bf16 priming matmuls (on a memset tile, no DMA dep)
    warm the PE HAM clock gate before real work lands (cold PE = 1.2 GHz).
  * Spatial phase PER SAMPLE on a [64, 64] layout: sample 0's epilogue
    hides completely under sample 1's DMA/compute; only sample 1's short
    chain is exposed.  Vertical 3-tap via tridiagonal matmul, horizontal
    via free-dim shifted adds, rsqrt via magic-seed Newton (1 round,
    ~2e-3 max rel err) so the whole kernel uses ONE ACT table set
    (exp_and_others), loaded once at kernel start.
  * Mask pre-scaled on host (mask * 1e30) and packed into the constant
    tensor; softmax denominators via tiny indicator matmuls.
"""

import sys

sys.path.insert(0, "/opt/trn_rl_repo")

import numpy as np

import concourse.bacc as bacc
import concourse.mybir as mybir
import concourse.tile as tile
from concourse.bass_utils import run_bass_kernel_spmd

B, C, H, W = 16, 256, 64, 64
N = H * W
NCORES = 8
SPC = B // NCORES  # samples per core
FP32 = mybir.dt.float32
FP32R = mybir.dt.float32r
BF16 = mybir.dt.bfloat16
I32 = mybir.dt.int32

AF = mybir.ActivationFunctionType
ALU = mybir.AluOpType

# Const tensor column layout (see const_base()).
CB_BAND = 0       # [:, 0:15]    sliding indicator band
CB_BAND2 = 16     # [0:64, 16:80]  tridiagonal T64 for the vertical 3-tap
CB_SEL2 = 144     # [:, 144:146] per-sample selector [128, 2]
CB_SELB2 = 160    # [0:2, 160:288] broadcast selector [2, 128]
CB_MASK = 320     # [:, 320:384] mask * 1e30 in [128, 64] pair layout
CB_COLS = 384

HC = 2048  # pixels per half-chunk DMA
MAGIC = 0x5F3759DF  # rsqrt bit-trick seed


def _r(ap):
    return ap.bitcast(FP32R)


def _i(ap):
    return ap.bitcast(I32)


def _kernel_body(ctx, tc, x, consts, out):
    nc = tc.nc

    cpool = ctx.enter_context(tc.tile_pool(name="consts", bufs=1))
    xp = ctx.enter_context(tc.tile_pool(name="xp", bufs=8))
    sqp = ctx.enter_context(tc.tile_pool(name="sqp", bufs=8))
    rows = ctx.enter_context(tc.tile_pool(name="rows", bufs=2))
    sp = ctx.enter_context(tc.tile_pool(name="sp", bufs=1))
    psa = ctx.enter_context(tc.tile_pool(name="psa", bufs=2, space="PSUM"))
    pss = ctx.enter_context(tc.tile_pool(name="pss", bufs=1, space="PSUM"))

    # Constants + pre-scaled mask in one DMA (f32r-typed so the verifier
    # accepts the band slices as rounded fp32r matmul inputs).
    CT = cpool.tile([128, CB_COLS], FP32)
    nc.sync.dma_start(out=_r(CT[:]), in_=_r(consts.ap()))

    # All eight 1 MiB x half-chunk loads: k=0 on the sync HWDGE ring,
    # k=1 on the scalar ring, sample 0 first on both.
    xt = {}
    for s in range(SPC):
        for k in range(2):
            for h in range(2):
                t = xp.tile([128, HC], FP32, tag="x")
                eng = nc.sync if k == 0 else nc.scalar
                eng.dma_start(
                    out=_r(t[:]),
                    in_=_r(x[s, 128 * k : 128 * (k + 1), HC * h : HC * (h + 1)]),
                )
                xt[(s, k, h)] = t

    # Warm the single ACT table set (exp_and_others: exp/square/copy).
    warm = sp.tile([1, 4], FP32, tag="warm")
    nc.vector.memset(warm[:], 1.0)
    wo = sp.tile([1, 4], FP32, tag="warmout")
    nc.scalar.activation(wo[0:1, 0:2], warm[0:1, 0:2], AF.Exp)

    # bf16 copy of the sliding band for the sum-of-squares matmuls.
    bandb = cpool.tile([128, 16], BF16)
    nc.vector.tensor_copy(bandb[:, 0:15], CT[:, 0:15])

    # Zero-padded horizontal-shift tile (pair layout).
    Hb = sp.tile([128, 66], FP32, tag="Hb")
    nc.vector.memset(Hb[:], 0.0)
    Sb2 = sp.tile([128, 64], FP32, tag="Sb2")
    Qt2 = sp.tile([128, 64], FP32, tag="Qt2")

    # Prime the PE HAM clock gate while DMAs fill: ~5us of dummy bf16
    # matmul activity lifts the PE from 1.2 to 2.4 GHz before real work.
    pr = cpool.tile([128, 512], BF16)
    nc.gpsimd.memset(pr[:], 0.0)
    prime_ps = pss.tile([8, 512], FP32, tag="prime")
    for i in range(16):
        nc.tensor.matmul(
            prime_ps[:], pr[:, 0:8], pr[:], start=i == 0, stop=i == 15
        )

    # Channel reductions: sum and sum-of-squares per pixel, [8, 512] psum
    # (row g = pixel block g), reshaped to [64, 64] per sample (partition
    # = image row), then the spatial phase runs per sample so sample 0's
    # epilogue hides under sample 1's compute.
    for s in range(SPC):
        ps_s = psa.tile([8, 512], FP32, tag="ps_s")
        ps_q = psa.tile([8, 512], FP32, tag="ps_q")
        # Chunks in expected DMA-arrival order (the two rings progress in
        # parallel), so the PE FIFO never stalls on a not-yet-landed piece
        # while a landed one waits behind it.  Each piece's square runs as
        # two 1024-pixel halves on ACT and DVE in parallel.
        for ci, (k, h) in enumerate([(1, 0), (0, 0), (1, 1), (0, 1)]):
            t = xt[(s, k, h)]
            sq = sqp.tile([128, HC], BF16, tag="sq")
            eng_a, eng_b = (nc.scalar, nc.vector) if k == 0 else (nc.vector, nc.scalar)
            if eng_a is nc.scalar:
                nc.scalar.activation(sq[:, 0:1024], t[:, 0:1024], AF.Square)
                nc.vector.tensor_mul(sq[:, 1024:2048], t[:, 1024:2048], t[:, 1024:2048])
            else:
                nc.vector.tensor_mul(sq[:, 0:1024], t[:, 0:1024], t[:, 0:1024])
                nc.scalar.activation(sq[:, 1024:2048], t[:, 1024:2048], AF.Square)
            for l in range(4):
                g = 4 * h + l
                nc.tensor.matmul(
                    ps_s[:],
                    _r(CT[:, CB_BAND + 7 - g : CB_BAND + 15 - g]),
                    _r(t[:, 512 * l : 512 * (l + 1)]),
                    start=ci == 0 and l == 0, stop=ci == 3 and l == 3,
                )
            for l in range(4):
                g = 4 * h + l
                nc.tensor.matmul(
                    ps_q[:],
                    bandb[:, 7 - g : 15 - g],
                    sq[:, 512 * l : 512 * (l + 1)],
                    start=ci == 0 and l == 0, stop=ci == 3 and l == 3,
                )
        s_sb = rows.tile([8, 512], FP32, tag="srow")
        q_sb = rows.tile([8, 512], FP32, tag="qrow")
        nc.scalar.copy(s_sb[:], ps_s[:])
        nc.vector.tensor_copy(q_sb[:], ps_q[:])
        # [8, 512] -> rows 64s..64s+63 of the [128, 64] pair tiles (both
        # APs enumerate pixels in order).  SWDGE queue: never waits behind
        # the big HWDGE loads.
        nc.gpsimd.dma_start(out=Sb2[64 * s : 64 * (s + 1), :], in_=s_sb[:])
        nc.gpsimd.dma_start(out=Qt2[64 * s : 64 * (s + 1), :], in_=q_sb[:])

    # --- pair-batched spatial phase: one ~16-op chain on [128, 64] tiles
    # (same per-op cost as [64, 64] but half the op count and a single
    # serial chain, so no FIFO head-of-line blocking between samples). ---
    v_ps = pss.tile([128, 64], FP32, tag="vps")
    nc.tensor.matmul(
        v_ps[:], CT[:, CB_BAND2 : CB_BAND2 + 128], Sb2[:],
        start=True, stop=True,
    )
    nc.scalar.copy(Hb[:, 1:65], v_ps[:])
    T1 = sp.tile([128, 64], FP32)
    nc.vector.tensor_add(T1[:], Hb[:, 0:64], Hb[:, 1:65])
    BOX = sp.tile([128, 64], FP32)
    nc.vector.tensor_add(BOX[:], T1[:], Hb[:, 2:66])

    # sim = (box*s) / sqrt(max((16/9*box)^2, 1e-12) * q).  The eps clamp
    # rides on box^2 alone: q >= O(100) always, so the reference's product
    # clamp binds iff this one does (and only where sim ~ 0 anyway).
    P = sp.tile([128, 64], FP32)
    nc.scalar.activation(P[:], BOX[:], AF.Square, scale=16.0 / 9.0)
    T = sp.tile([128, 64], FP32)
    nc.vector.tensor_mul(T[:], BOX[:], Sb2[:])
    Dt = sp.tile([128, 64], FP32)
    nc.vector.scalar_tensor_tensor(
        Dt[:], P[:], 1e-12, Qt2[:], op0=ALU.max, op1=ALU.mult
    )

    # R = Dt^-1/2 via magic-seed Newton (1 round, ~2e-3 rel err -- the
    # tolerance is 2e-2): y0 = bitcast(MAGIC - (bitcast(Dt) >> 1)).
    nt = sp.tile([128, 64], FP32)
    nc.vector.tensor_scalar(
        _i(nt[:]), _i(Dt[:]), 1, -1,
        op0=ALU.logical_shift_right, op1=ALU.bitwise_xor,
    )
    y0 = sp.tile([128, 64], FP32)
    nc.vector.tensor_scalar(
        _i(y0[:]), _i(nt[:]), MAGIC + 1, None, op0=ALU.add
    )
    a = sp.tile([128, 64], FP32)
    nc.vector.tensor_mul(a[:], y0[:], y0[:])
    hh = sp.tile([128, 64], FP32)
    nc.vector.scalar_tensor_tensor(
        hh[:], Dt[:], 0.5, a[:], op0=ALU.mult, op1=ALU.mult
    )
    m1 = sp.tile([128, 64], FP32)
    nc.vector.scalar_tensor_tensor(
        m1[:], hh[:], -1.0, y0[:], op0=ALU.mult, op1=ALU.mult
    )
    y = sp.tile([128, 64], FP32)
    nc.vector.scalar_tensor_tensor(
        y[:], y0[:], 1.5, m1[:], op0=ALU.mult, op1=ALU.add
    )

    # U = box*s*R; EM = exp(-(U + 1e30*mask)/9) = masked exp(-sim), with
    # per-row sums accumulated for free by the ACT op.
    U = sp.tile([128, 64], FP32)
    nc.vector.tensor_mul(U[:], T[:], y[:])
    U2 = sp.tile([128, 64], FP32)
    nc.vector.tensor_add(U2[:], U[:], CT[:, CB_MASK : CB_MASK + 64])
    EM = sp.tile([128, 64], FP32)
    rowsum = sp.tile([128, 1], FP32)
    nc.scalar.activation(
        EM[:], U2[:], AF.Exp, scale=-1.0 / 9.0, accum_out=rowsum[:]
    )

    # Per-sample totals and broadcast back via tiny indicator matmuls.
    tot_ps = pss.tile([2, 1], FP32, tag="tot")
    nc.tensor.matmul(
        tot_ps[:], CT[:, CB_SEL2 : CB_SEL2 + 2], rowsum[:],
        start=True, stop=True,
    )
    rec = sp.tile([2, 1], FP32)
    nc.vector.reciprocal(rec[:], tot_ps[:])
    recb_ps = pss.tile([128, 1], FP32, tag="recb")
    nc.tensor.matmul(
        recb_ps[:], CT[0:2, CB_SELB2 : CB_SELB2 + 128], rec[:],
        start=True, stop=True,
    )
    OUTt = sp.tile([128, 64], FP32)
    nc.vector.tensor_scalar_mul(OUTt[:], EM[:], recb_ps[:, 0:1])
    nc.sync.dma_start(
        out=out.ap().rearrange("s (r c) -> (s r) c", c=64), in_=OUTt[:]
    )


_NC_CACHE = {}


def _build():
    key = "v6"
    if key in _NC_CACHE:
        return _NC_CACHE[key]
    nc = bacc.Bacc("TRN2", target_bir_lowering=False, debug=False)
    x = nc.declare_dram_parameter("x", [SPC, C, N], FP32, isOutput=False)
    consts = nc.declare_dram_parameter("consts", [128, CB_COLS], FP32, isOutput=False)
    out = nc.declare_dram_parameter("out", [SPC, N], FP32, isOutput=True)
    from contextlib import ExitStack

    with tile.TileContext(nc) as tc, ExitStack() as ctx:
        _kernel_body(ctx, tc, x, consts, out)
    nc.compile()
    _NC_CACHE[key] = nc
    return nc


def const_base() -> np.ndarray:
    ct = np.zeros((128, CB_COLS), dtype=np.float32)
    # Sliding indicator band: column 7 all-ones; slice [:, 7-g:15-g] puts
    # the ones-column at position g.
    ct[:, CB_BAND + 7] = 1.0
    # Block-diagonal tridiagonal for the vertical 3-tap (both samples).
    idx = np.arange(64)
    t64 = (np.abs(idx[:, None] - idx[None, :]) <= 1).astype(np.float32)
    ct[0:64, CB_BAND2 : CB_BAND2 + 64] = t64
    ct[64:128, CB_BAND2 + 64 : CB_BAND2 + 128] = t64
    # Per-sample selectors for the softmax total + broadcast.
    ct[0:64, CB_SEL2] = 1.0
    ct[64:128, CB_SEL2 + 1] = 1.0
    ct[0, CB_SELB2 : CB_SELB2 + 64] = 1.0
    ct[1, CB_SELB2 + 64 : CB_SELB2 + 128] = 1.0
    return ct


_CT_BASE = const_base()


def make_in_maps(x: np.ndarray, prev_drop_mask: np.ndarray) -> list:
    xs = np.ascontiguousarray(np.asarray(x), dtype=np.float32).reshape(B, C, N)
    mb = (np.asarray(prev_drop_mask).astype(np.float32) * 1e30).reshape(B, H, W)
    in_maps = []
    for i in range(NCORES):
        ct = _CT_BASE.copy()
        ct[0:64, CB_MASK : CB_MASK + 64] = mb[2 * i]
        ct[64:128, CB_MASK : CB_MASK + 64] = mb[2 * i + 1]
        in_maps.append({"x": xs[i * SPC : (i + 1) * SPC], "consts": ct})
    return in_maps


def kernel(x: np.ndarray, prev_drop_mask: np.ndarray) -> np.ndarray:
    nc = _build()
    res = run_bass_kernel_spmd(nc, make_in_maps(x, prev_drop_mask), list(range(NCORES)))
    outs = [res.results[i]["out"] for i in range(NCORES)]
    return np.concatenate(outs, axis=0).reshape(B, H, W)


# revision 17
# speedup vs baseline: 1.0183x; 1.0183x over previous
"""Trainium2 Bass kernel for LocalSpatialSimilarity.

Per sample (B=16, C=256, H=W=64, N=4096 pixels):
  s[p]  = sum_c x[c,p]                (channel sum)
  q[p]  = sum_c x[c,p]^2              (channel sum of squares)
  box   = 3x3 zero-padded box-sum of s (reshaped to 64x64)
  sim   = (box/9 * s) / sqrt(max(q * box^2 * 256/81, 1e-12))
  out   = softmax over p of (mask ? -inf : -sim)
        = (mask ? 0 : exp(-sim)) / total        (sim bounded in [-1,1] -> no
                                                 max-subtraction needed)

Sharding: pure data parallel, 2 samples per core across 8 cores.

v4 design (evolved through perfetto-trace iterations):
  * x streams in eight 1 MiB half-chunks over BOTH HWDGE rings (channel
    chunk 0 on sync, chunk 1 on scalar) -- measured ~430 GB/s aggregate.
    SWDGE proved ~150 GB/s, so it only carries the tiny reshape/out DMAs
    (where it avoids FIFO-queueing behind the big loads).
  * Channel reductions on the PE: fp32r for sum(x) (DMA writes through
    f32r-typed APs so the BIR verifier sees rounded producers), bf16 for
    sum(x^2).  Sliding 8-wide indicator band stationaries land pixel
    block g on psum partition g; s-matmuls and q-matmuls are emitted in
    separate bursts so a pending square never blocks ready s-matmuls in
    the PE FIFO.
  * Squares split between ACT (chunk 0) and DVE (chunk 1) so neither
    queue stalls, and so DMA-issue slots never sit behind a 2us square.
  * ~5us of dummy bf16 priming matmuls (on a memset tile, no DMA dep)
    warm the PE HAM clock gate before real work lands (cold PE = 1.2 GHz).
  * Spatial phase PER SAMPLE on a [64, 64] layout: sample 0's epilogue
    hides completely under sample 1's DMA/compute; only sample 1's short
    chain is exposed.  Vertical 3-tap via tridiagonal matmul, horizontal
    via free-dim shifted adds, rsqrt via magic-seed Newton (1 round,
    ~2e-3 max rel err) so the whole kernel uses ONE ACT table set
    (exp_and_others), loaded once at kernel start.
  * Mask pre-scaled on host (mask * 1e30) and packed into the constant
    tensor; softmax denominators via tiny indicator matmuls.
"""

import sys

sys.path.insert(0, "/opt/trn_rl_repo")

import numpy as np

import concourse.bacc as bacc
import concourse.mybir as mybir
import concourse.tile as tile
from concourse.bass_utils import run_bass_kernel_spmd

B, C, H, W = 16, 256, 64, 64
N = H * W
NCORES = 8
SPC = B // NCORES  # samples per core
FP32 = mybir.dt.float32
FP32R = mybir.dt.float32r
BF16 = mybir.dt.bfloat16
I32 = mybir.dt.int32

AF = mybir.ActivationFunctionType
ALU = mybir.AluOpType

# Const tensor column layout (see const_base()).
CB_BAND = 0       # [:, 0:15]    sliding indicator band
CB_BAND2 = 16     # [0:64, 16:80]  tridiagonal T64 for the vertical 3-tap
CB_SEL2 = 144     # [:, 144:146] per-sample selector [128, 2]
CB_SELB2 = 160    # [0:2, 160:288] broadcast selector [2, 128]
CB_MASK = 320     # [:, 320:384] mask * 1e30 in [128, 64] pair layout
CB_COLS = 384

HC = 2048  # pixels per half-chunk DMA
MAGIC = 0x5F3759DF  # rsqrt bit-trick seed


def _r(ap):
    return ap.bitcast(FP32R)


def _i(ap):
    return ap.bitcast(I32)


def _kernel_body(ctx, tc, x, consts, out):
    nc = tc.nc

    cpool = ctx.enter_context(tc.tile_pool(name="consts", bufs=1))
    xp = ctx.enter_context(tc.tile_pool(name="xp", bufs=8))
    sqp = ctx.enter_context(tc.tile_pool(name="sqp", bufs=8))
    rows = ctx.enter_context(tc.tile_pool(name="rows", bufs=2))
    sp = ctx.enter_context(tc.tile_pool(name="sp", bufs=1))
    psa = ctx.enter_context(tc.tile_pool(name="psa", bufs=2, space="PSUM"))
    pss = ctx.enter_context(tc.tile_pool(name="pss", bufs=1, space="PSUM"))

    # Constants + pre-scaled mask in one DMA (f32r-typed so the verifier
    # accepts the band slices as rounded fp32r matmul inputs).
    CT = cpool.tile([128, CB_COLS], FP32)
    nc.sync.dma_start(out=_r(CT[:]), in_=_r(consts.ap()))

    # All eight 1 MiB x half-chunk loads: k=0 on the sync HWDGE ring,
    # k=1 on the scalar ring, sample 0 first on both.
    xt = {}
    for s in range(SPC):
        for k in range(2):
            for h in range(2):
                t = xp.tile([128, HC], FP32, tag="x")
                eng = nc.sync if k == 0 else nc.scalar
                eng.dma_start(
                    out=_r(t[:]),
                    in_=_r(x[s, 128 * k : 128 * (k + 1), HC * h : HC * (h + 1)]),
                )
                xt[(s, k, h)] = t

    # Warm the single ACT table set (exp_and_others: exp/square/copy).
    warm = sp.tile([1, 4], FP32, tag="warm")
    nc.vector.memset(warm[:], 1.0)
    wo = sp.tile([1, 4], FP32, tag="warmout")
    nc.scalar.activation(wo[0:1, 0:2], warm[0:1, 0:2], AF.Exp)

    # bf16 copy of the sliding band for the sum-of-squares matmuls.
    bandb = cpool.tile([128, 16], BF16)
    nc.vector.tensor_copy(bandb[:, 0:15], CT[:, 0:15])

    # Zero-padded horizontal-shift tile (pair layout).
    Hb = sp.tile([128, 66], FP32, tag="Hb")
    nc.vector.memset(Hb[:], 0.0)
    Sb2 = sp.tile([128, 64], FP32, tag="Sb2")
    Qt2 = sp.tile([128, 64], FP32, tag="Qt2")

    # Prime the PE HAM clock gate while DMAs fill: ~5us of dummy bf16
    # matmul activity lifts the PE from 1.2 to 2.4 GHz before real work.
    pr = cpool.tile([128, 512], BF16)
    nc.gpsimd.memset(pr[:], 0.0)
    prime_ps = pss.tile([8, 512], FP32, tag="prime")
    for i in range(16):
        nc.tensor.matmul(
            prime_ps[:], pr[:, 0:8], pr[:], start=i == 0, stop=i == 15
        )

    # Channel reductions: sum and sum-of-squares per pixel, [8, 512] psum
    # (row g = pixel block g), reshaped to [64, 64] per sample (partition
    # = image row), then the spatial phase runs per sample so sample 0's
    # epilogue hides under sample 1's compute.
    for s in range(SPC):
        ps_s = psa.tile([8, 512], FP32, tag="ps_s")
        ps_q = psa.tile([8, 512], FP32, tag="ps_q")
        # Chunks in expected DMA-arrival order (the two rings progress in
        # parallel), so the PE FIFO never stalls on a not-yet-landed piece
        # while a landed one waits behind it.  Each piece's square runs as
        # two 1024-pixel halves on ACT and DVE in parallel.
        for ci, (k, h) in enumerate([(1, 0), (0, 0), (1, 1), (0, 1)]):
            t = xt[(s, k, h)]
            sq = sqp.tile([128, HC], BF16, tag="sq")
            eng_a, eng_b = (nc.scalar, nc.vector) if k == 0 else (nc.vector, nc.scalar)
            if eng_a is nc.scalar:
                nc.scalar.activation(sq[:, 0:1024], t[:, 0:1024], AF.Square)
                nc.vector.tensor_mul(sq[:, 1024:2048], t[:, 1024:2048], t[:, 1024:2048])
            else:
                nc.vector.tensor_mul(sq[:, 0:1024], t[:, 0:1024], t[:, 0:1024])
                nc.scalar.activation(sq[:, 1024:2048], t[:, 1024:2048], AF.Square)
            for l in range(4):
                g = 4 * h + l
                nc.tensor.matmul(
                    ps_s[:],
                    _r(CT[:, CB_BAND + 7 - g : CB_BAND + 15 - g]),
                    _r(t[:, 512 * l : 512 * (l + 1)]),
                    start=ci == 0 and l == 0, stop=ci == 3 and l == 3,
                )
            for l in range(4):
                g = 4 * h + l
                nc.tensor.matmul(
                    ps_q[:],
                    bandb[:, 7 - g : 15 - g],
                    sq[:, 512 * l : 512 * (l + 1)],
                    start=ci == 0 and l == 0, stop=ci == 3 and l == 3,
                )
        s_sb = rows.tile([8, 512], FP32, tag="srow")
        q_sb = rows.tile([8, 512], FP32, tag="qrow")
        nc.scalar.copy(s_sb[:], ps_s[:])
        nc.scalar.copy(q_sb[:], ps_q[:])
        # [8, 512] -> rows 64s..64s+63 of the [128, 64] pair tiles (both
        # APs enumerate pixels in order).  SWDGE queue: never waits behind
        # the big HWDGE loads.
        reng = nc.gpsimd if s == 0 else nc.sync
        reng.dma_start(out=Sb2[64 * s : 64 * (s + 1), :], in_=s_sb[:])
        reng.dma_start(out=Qt2[64 * s : 64 * (s + 1), :], in_=q_sb[:])

    # --- pair-batched spatial phase: one ~16-op chain on [128, 64] tiles
    # (same per-op cost as [64, 64] but half the op count and a single
    # serial chain, so no FIFO head-of-line blocking between samples). ---
    v_ps = pss.tile([128, 64], FP32, tag="vps")
    nc.tensor.matmul(
        v_ps[:], CT[:, CB_BAND2 : CB_BAND2 + 128], Sb2[:],
        start=True, stop=True,
    )
    nc.scalar.copy(Hb[:, 1:65], v_ps[:])
    T1 = sp.tile([128, 64], FP32)
    nc.vector.tensor_add(T1[:], Hb[:, 0:64], Hb[:, 1:65])
    BOX = sp.tile([128, 64], FP32)
    nc.vector.tensor_add(BOX[:], T1[:], Hb[:, 2:66])

    # sim = (box*s) / sqrt(max((16/9*box)^2, 1e-12) * q).  The eps clamp
    # rides on box^2 alone: q >= O(100) always, so the reference's product
    # clamp binds iff this one does (and only where sim ~ 0 anyway).
    P = sp.tile([128, 64], FP32)
    nc.scalar.activation(P[:], BOX[:], AF.Square, scale=16.0 / 9.0)
    T = sp.tile([128, 64], FP32)
    nc.vector.tensor_mul(T[:], BOX[:], Sb2[:])
    Dt = sp.tile([128, 64], FP32)
    nc.vector.scalar_tensor_tensor(
        Dt[:], P[:], 1e-12, Qt2[:], op0=ALU.max, op1=ALU.mult
    )

    # R = Dt^-1/2 via magic-seed Newton (1 round, ~2e-3 rel err -- the
    # tolerance is 2e-2): y0 = bitcast(MAGIC - (bitcast(Dt) >> 1)).
    nt = sp.tile([128, 64], FP32)
    nc.vector.tensor_scalar(
        _i(nt[:]), _i(Dt[:]), 1, -1,
        op0=ALU.logical_shift_right, op1=ALU.bitwise_xor,
    )
    y0 = sp.tile([128, 64], FP32)
    nc.vector.tensor_scalar(
        _i(y0[:]), _i(nt[:]), MAGIC + 1, None, op0=ALU.add
    )
    a = sp.tile([128, 64], FP32)
    nc.vector.tensor_mul(a[:], y0[:], y0[:])
    hh = sp.tile([128, 64], FP32)
    nc.vector.scalar_tensor_tensor(
        hh[:], Dt[:], 0.5, a[:], op0=ALU.mult, op1=ALU.mult
    )
    m1 = sp.tile([128, 64], FP32)
    nc.vector.scalar_tensor_tensor(
        m1[:], hh[:], -1.0, y0[:], op0=ALU.mult, op1=ALU.mult
    )
    y = sp.tile([128, 64], FP32)
    nc.vector.scalar_tensor_tensor(
        y[:], y0[:], 1.5, m1[:], op0=ALU.mult, op1=ALU.add
    )

    # U = box*s*R; EM = exp(-(U + 1e30*mask)/9) = masked exp(-sim), with
    # per-row sums accumulated for free by the ACT op.
    U = sp.tile([128, 64], FP32)
    nc.vector.tensor_mul(U[:], T[:], y[:])
    U2 = sp.tile([128, 64], FP32)
    nc.vector.tensor_add(U2[:], U[:], CT[:, CB_MASK : CB_MASK + 64])
    EM = sp.tile([128, 64], FP32)
    rowsum = sp.tile([128, 1], FP32)
    nc.scalar.activation(
        EM[:], U2[:], AF.Exp, scale=-1.0 / 9.0, accum_out=rowsum[:]
    )

    # Per-sample totals and broadcast back via tiny indicator matmuls.
    tot_ps = pss.tile([2, 1], FP32, tag="tot")
    nc.tensor.matmul(
        tot_ps[:], CT[:, CB_SEL2 : CB_SEL2 + 2], rowsum[:],
        start=True, stop=True,
    )
    rec = sp.tile([2, 1], FP32)
    nc.vector.reciprocal(rec[:], tot_ps[:])
    recb_ps = pss.tile([128, 1], FP32, tag="recb")
    nc.tensor.matmul(
        recb_ps[:], CT[0:2, CB_SELB2 : CB_SELB2 + 128], rec[:],
        start=True, stop=True,
    )
    OUTt = sp.tile([128, 64], FP32)
    nc.vector.tensor_scalar_mul(OUTt[:], EM[:], recb_ps[:, 0:1])
    nc.sync.dma_start(
        out=out.ap().rearrange("s (r c) -> (s r) c", c=64), in_=OUTt[:]
    )


_NC_CACHE = {}


def _build():
    key = "v7"
    if key in _NC_CACHE:
        return _NC_CACHE[key]
    nc = bacc.Bacc("TRN2", target_bir_lowering=False, debug=False)
    x = nc.declare_dram_parameter("x", [SPC, C, N], FP32, isOutput=False)
    consts = nc.declare_dram_parameter("consts", [128, CB_COLS], FP32, isOutput=False)
    out = nc.declare_dram_parameter("out", [SPC, N], FP32, isOutput=True)
    from contextlib import ExitStack

    with tile.TileContext(nc) as tc, ExitStack() as ctx:
        _kernel_body(ctx, tc, x, consts, out)
    nc.compile()
    _NC_CACHE[key] = nc
    return nc


def const_base() -> np.ndarray:
    ct = np.zeros((128, CB_COLS), dtype=np.float32)
    # Sliding indicator band: column 7 all-ones; slice [:, 7-g:15-g] puts
    # the ones-column at position g.
    ct[:, CB_BAND + 7] = 1.0
    # Block-diagonal tridiagonal for the vertical 3-tap (both samples).
    idx = np.arange(64)
    t64 = (np.abs(idx[:, None] - idx[None, :]) <= 1).astype(np.float32)
    ct[0:64, CB_BAND2 : CB_BAND2 + 64] = t64
    ct[64:128, CB_BAND2 + 64 : CB_BAND2 + 128] = t64
    # Per-sample selectors for the softmax total + broadcast.
    ct[0:64, CB_SEL2] = 1.0
    ct[64:128, CB_SEL2 + 1] = 1.0
    ct[0, CB_SELB2 : CB_SELB2 + 64] = 1.0
    ct[1, CB_SELB2 + 64 : CB_SELB2 + 128] = 1.0
    return ct


_CT_BASE = const_base()


def make_in_maps(x: np.ndarray, prev_drop_mask: np.ndarray) -> list:
    xs = np.ascontiguousarray(np.asarray(x), dtype=np.float32).reshape(B, C, N)
    mb = (np.asarray(prev_drop_mask).astype(np.float32) * 1e30).reshape(B, H, W)
    in_maps = []
    for i in range(NCORES):
        ct = _CT_BASE.copy()
        ct[0:64, CB_MASK : CB_MASK + 64] = mb[2 * i]
        ct[64:128, CB_MASK : CB_MASK + 64] = mb[2 * i + 1]
        in_maps.append({"x": xs[i * SPC : (i + 1) * SPC], "consts": ct})
    return in_maps


def kernel(x: np.ndarray, prev_drop_mask: np.ndarray) -> np.ndarray:
    nc = _build()
    res = run_bass_kernel_spmd(nc, make_in_maps(x, prev_drop_mask), list(range(NCORES)))
    outs = [res.results[i]["out"] for i in range(NCORES)]
    return np.concatenate(outs, axis=0).reshape(B, H, W)


# revision 18
# speedup vs baseline: 1.0998x; 1.0800x over previous
"""Trainium2 Bass kernel for LocalSpatialSimilarity.

Per sample (B=16, C=256, H=W=64, N=4096 pixels):
  s[p]  = sum_c x[c,p]                (channel sum)
  q[p]  = sum_c x[c,p]^2              (channel sum of squares)
  box   = 3x3 zero-padded box-sum of s (reshaped to 64x64)
  sim   = (box/9 * s) / sqrt(max(q * box^2 * 256/81, 1e-12))
  out   = softmax over p of (mask ? -inf : -sim)
        = (mask ? 0 : exp(-sim)) / total        (sim bounded in [-1,1] -> no
                                                 max-subtraction needed)

Sharding: pure data parallel, 2 samples per core across 8 cores.

v4 design (evolved through perfetto-trace iterations):
  * x streams in eight 1 MiB half-chunks over BOTH HWDGE rings (channel
    chunk 0 on sync, chunk 1 on scalar) -- measured ~430 GB/s aggregate.
    SWDGE proved ~150 GB/s, so it only carries the tiny reshape/out DMAs
    (where it avoids FIFO-queueing behind the big loads).
  * Channel reductions on the PE: fp32r for sum(x) (DMA writes through
    f32r-typed APs so the BIR verifier sees rounded producers), bf16 for
    sum(x^2).  Sliding 8-wide indicator band stationaries land pixel
    block g on psum partition g; s-matmuls and q-matmuls are emitted in
    separate bursts so a pending square never blocks ready s-matmuls in
    the PE FIFO.
  * Squares split between ACT (chunk 0) and DVE (chunk 1) so neither
    queue stalls, and so DMA-issue slots never sit behind a 2us square.
  * ~5us of dummy bf16 priming matmuls (on a memset tile, no DMA dep)
    warm the PE HAM clock gate before real work lands (cold PE = 1.2 GHz).
  * Spatial phase PER SAMPLE on a [64, 64] layout: sample 0's epilogue
    hides completely under sample 1's DMA/compute; only sample 1's short
    chain is exposed.  Vertical 3-tap via tridiagonal matmul, horizontal
    via free-dim shifted adds, rsqrt via magic-seed Newton (1 round,
    ~2e-3 max rel err) so the whole kernel uses ONE ACT table set
    (exp_and_others), loaded once at kernel start.
  * Mask pre-scaled on host (mask * 1e30) and packed into the constant
    tensor; softmax denominators via tiny indicator matmuls.
"""

import sys

sys.path.insert(0, "/opt/trn_rl_repo")

import numpy as np

import concourse.bacc as bacc
import concourse.mybir as mybir
import concourse.tile as tile
from concourse.bass_utils import run_bass_kernel_spmd

B, C, H, W = 16, 256, 64, 64
N = H * W
NCORES = 8
SPC = B // NCORES  # samples per core
FP32 = mybir.dt.float32
FP32R = mybir.dt.float32r
BF16 = mybir.dt.bfloat16
I32 = mybir.dt.int32

AF = mybir.ActivationFunctionType
ALU = mybir.AluOpType

# Const tensor column layout (see const_base()).
CB_BAND = 0       # [:, 0:15]    sliding indicator band
CB_BAND2 = 16     # [0:64, 16:80]  tridiagonal T64 for the vertical 3-tap
CB_SEL2 = 144     # [:, 144:146] per-sample selector [128, 2]
CB_SELB2 = 160    # [0:2, 160:288] broadcast selector [2, 128]
CB_MASK = 320     # [:, 320:384] mask * 1e30 in [128, 64] pair layout
CB_COLS = 384

HC = 2048  # pixels per half-chunk DMA
MAGIC = 0x5F3759DF  # rsqrt bit-trick seed


def _r(ap):
    return ap.bitcast(FP32R)


def _i(ap):
    return ap.bitcast(I32)


def _kernel_body(ctx, tc, x, consts, out):
    nc = tc.nc

    cpool = ctx.enter_context(tc.tile_pool(name="consts", bufs=1))
    xp = ctx.enter_context(tc.tile_pool(name="xp", bufs=12))
    sqp = ctx.enter_context(tc.tile_pool(name="sqp", bufs=12))
    rows = ctx.enter_context(tc.tile_pool(name="rows", bufs=2))
    sp = ctx.enter_context(tc.tile_pool(name="sp", bufs=1))
    psa = ctx.enter_context(tc.tile_pool(name="psa", bufs=2, space="PSUM"))
    pss = ctx.enter_context(tc.tile_pool(name="pss", bufs=1, space="PSUM"))

    # Constants + pre-scaled mask in one DMA (f32r-typed so the verifier
    # accepts the band slices as rounded fp32r matmul inputs).
    CT = cpool.tile([128, CB_COLS], FP32)
    nc.gpsimd.dma_start(out=_r(CT[:]), in_=_r(consts.ap()))

    # x loads: k=0 on the sync HWDGE ring, k=1 on the scalar ring, pure x
    # on both rings (small DMAs all ride SWDGE so the rings stay symmetric
    # and at full descriptor size).  Pieces taper (2048/1536/512 pixels)
    # so the work gated on the final piece is small.
    PIECES = [(0, 2048), (2048, 1536), (3584, 512)]
    xt = {}
    for s in range(SPC):
        for k in range(2):
            for pi, (p0, pn) in enumerate(PIECES):
                t = xp.tile([128, 2048], FP32, tag="x")
                eng = nc.sync if k == 0 else nc.scalar
                eng.dma_start(
                    out=_r(t[:, 0:pn]),
                    in_=_r(x[s, 128 * k : 128 * (k + 1), p0 : p0 + pn]),
                )
                xt[(s, k, pi)] = t

    # Warm the single ACT table set (exp_and_others: exp/square/copy).
    warm = sp.tile([1, 4], FP32, tag="warm")
    nc.vector.memset(warm[:], 1.0)
    wo = sp.tile([1, 4], FP32, tag="warmout")
    nc.scalar.activation(wo[0:1, 0:2], warm[0:1, 0:2], AF.Exp)

    # bf16 copy of the sliding band for the sum-of-squares matmuls.
    bandb = cpool.tile([128, 16], BF16)
    nc.vector.tensor_copy(bandb[:, 0:15], CT[:, 0:15])

    # Zero-padded horizontal-shift tile (pair layout).
    Hb = sp.tile([128, 66], FP32, tag="Hb")
    nc.vector.memset(Hb[:], 0.0)
    Sb2 = sp.tile([128, 64], FP32, tag="Sb2")
    Qt2 = sp.tile([128, 64], FP32, tag="Qt2")

    # Prime the PE HAM clock gate while DMAs fill: ~5us of dummy bf16
    # matmul activity lifts the PE from 1.2 to 2.4 GHz before real work.
    pr = cpool.tile([128, 512], BF16)
    nc.gpsimd.memset(pr[:], 0.0)
    prime_ps = pss.tile([8, 512], FP32, tag="prime")
    for i in range(16):
        nc.tensor.matmul(
            prime_ps[:], pr[:, 0:8], pr[:], start=i == 0, stop=i == 15
        )

    # Channel reductions: sum and sum-of-squares per pixel, [8, 512] psum
    # (row g = pixel block g), reshaped to [64, 64] per sample (partition
    # = image row), then the spatial phase runs per sample so sample 0's
    # epilogue hides under sample 1's compute.
    for s in range(SPC):
        ps_s = psa.tile([8, 512], FP32, tag="ps_s")
        ps_q = psa.tile([8, 512], FP32, tag="ps_q")
        # Pieces in expected DMA-arrival order (the two rings progress in
        # parallel), so the PE FIFO never stalls on a not-yet-landed piece
        # while a landed one waits behind it.  Each piece's square runs as
        # two halves on ACT and DVE in parallel.
        order = [(k, pi) for pi in range(3) for k in (1, 0)]
        for ci, (k, pi) in enumerate(order):
            p0, pn = PIECES[pi]
            t = xt[(s, k, pi)]
            sq = sqp.tile([128, 2048], BF16, tag="sq")
            half = pn // 2
            if k == 0:
                nc.scalar.activation(sq[:, 0:half], t[:, 0:half], AF.Square)
                nc.vector.tensor_mul(sq[:, half:pn], t[:, half:pn], t[:, half:pn])
            else:
                nc.vector.tensor_mul(sq[:, 0:half], t[:, 0:half], t[:, 0:half])
                nc.scalar.activation(sq[:, half:pn], t[:, half:pn], AF.Square)
            nb = pn // 512
            for l in range(nb):
                g = p0 // 512 + l
                nc.tensor.matmul(
                    ps_s[:],
                    _r(CT[:, CB_BAND + 7 - g : CB_BAND + 15 - g]),
                    _r(t[:, 512 * l : 512 * (l + 1)]),
                    start=ci == 0 and l == 0, stop=ci == 5 and l == nb - 1,
                )
            for l in range(nb):
                g = p0 // 512 + l
                nc.tensor.matmul(
                    ps_q[:],
                    bandb[:, 7 - g : 15 - g],
                    sq[:, 512 * l : 512 * (l + 1)],
                    start=ci == 0 and l == 0, stop=ci == 5 and l == nb - 1,
                )
        s_sb = rows.tile([8, 512], FP32, tag="srow")
        q_sb = rows.tile([8, 512], FP32, tag="qrow")
        nc.scalar.copy(s_sb[:], ps_s[:])
        nc.scalar.copy(q_sb[:], ps_q[:])
        # [8, 512] -> rows 64s..64s+63 of the [128, 64] pair tiles (both
        # APs enumerate pixels in order).  SWDGE queue: never waits behind
        # the big HWDGE loads.
        nc.gpsimd.dma_start(out=Sb2[64 * s : 64 * (s + 1), :], in_=s_sb[:])
        nc.gpsimd.dma_start(out=Qt2[64 * s : 64 * (s + 1), :], in_=q_sb[:])

    # --- pair-batched spatial phase: one ~16-op chain on [128, 64] tiles
    # (same per-op cost as [64, 64] but half the op count and a single
    # serial chain, so no FIFO head-of-line blocking between samples). ---
    v_ps = pss.tile([128, 64], FP32, tag="vps")
    nc.tensor.matmul(
        v_ps[:], CT[:, CB_BAND2 : CB_BAND2 + 128], Sb2[:],
        start=True, stop=True,
    )
    nc.scalar.copy(Hb[:, 1:65], v_ps[:])
    T1 = sp.tile([128, 64], FP32)
    nc.vector.tensor_add(T1[:], Hb[:, 0:64], Hb[:, 1:65])
    BOX = sp.tile([128, 64], FP32)
    nc.vector.tensor_add(BOX[:], T1[:], Hb[:, 2:66])

    # sim = (box*s) / sqrt(max((16/9*box)^2, 1e-12) * q).  The eps clamp
    # rides on box^2 alone: q >= O(100) always, so the reference's product
    # clamp binds iff this one does (and only where sim ~ 0 anyway).
    P = sp.tile([128, 64], FP32)
    nc.scalar.activation(P[:], BOX[:], AF.Square, scale=16.0 / 9.0)
    T = sp.tile([128, 64], FP32)
    nc.vector.tensor_mul(T[:], BOX[:], Sb2[:])
    Dt = sp.tile([128, 64], FP32)
    nc.vector.scalar_tensor_tensor(
        Dt[:], P[:], 1e-12, Qt2[:], op0=ALU.max, op1=ALU.mult
    )

    # R = Dt^-1/2 via magic-seed Newton (1 round, ~2e-3 rel err -- the
    # tolerance is 2e-2): y0 = bitcast(MAGIC - (bitcast(Dt) >> 1)).
    nt = sp.tile([128, 64], FP32)
    nc.vector.tensor_scalar(
        _i(nt[:]), _i(Dt[:]), 1, -1,
        op0=ALU.logical_shift_right, op1=ALU.bitwise_xor,
    )
    y0 = sp.tile([128, 64], FP32)
    nc.vector.tensor_scalar(
        _i(y0[:]), _i(nt[:]), MAGIC + 1, None, op0=ALU.add
    )
    a = sp.tile([128, 64], FP32)
    nc.vector.tensor_mul(a[:], y0[:], y0[:])
    hh = sp.tile([128, 64], FP32)
    nc.vector.scalar_tensor_tensor(
        hh[:], Dt[:], 0.5, a[:], op0=ALU.mult, op1=ALU.mult
    )
    m1 = sp.tile([128, 64], FP32)
    nc.vector.scalar_tensor_tensor(
        m1[:], hh[:], -1.0, y0[:], op0=ALU.mult, op1=ALU.mult
    )
    y = sp.tile([128, 64], FP32)
    nc.vector.scalar_tensor_tensor(
        y[:], y0[:], 1.5, m1[:], op0=ALU.mult, op1=ALU.add
    )

    # U = box*s*R; EM = exp(-(U + 1e30*mask)/9) = masked exp(-sim), with
    # per-row sums accumulated for free by the ACT op.
    U = sp.tile([128, 64], FP32)
    nc.vector.tensor_mul(U[:], T[:], y[:])
    U2 = sp.tile([128, 64], FP32)
    nc.vector.tensor_add(U2[:], U[:], CT[:, CB_MASK : CB_MASK + 64])
    EM = sp.tile([128, 64], FP32)
    rowsum = sp.tile([128, 1], FP32)
    nc.scalar.activation(
        EM[:], U2[:], AF.Exp, scale=-1.0 / 9.0, accum_out=rowsum[:]
    )

    # Per-sample totals and broadcast back via tiny indicator matmuls.
    tot_ps = pss.tile([2, 1], FP32, tag="tot")
    nc.tensor.matmul(
        tot_ps[:], CT[:, CB_SEL2 : CB_SEL2 + 2], rowsum[:],
        start=True, stop=True,
    )
    rec = sp.tile([2, 1], FP32)
    nc.vector.reciprocal(rec[:], tot_ps[:])
    recb_ps = pss.tile([128, 1], FP32, tag="recb")
    nc.tensor.matmul(
        recb_ps[:], CT[0:2, CB_SELB2 : CB_SELB2 + 128], rec[:],
        start=True, stop=True,
    )
    OUTt = sp.tile([128, 64], FP32)
    nc.vector.tensor_scalar_mul(OUTt[:], EM[:], recb_ps[:, 0:1])
    nc.gpsimd.dma_start(
        out=out.ap().rearrange("s (r c) -> (s r) c", c=64), in_=OUTt[:]
    )


_NC_CACHE = {}


def _build():
    key = "v8"
    if key in _NC_CACHE:
        return _NC_CACHE[key]
    nc = bacc.Bacc("TRN2", target_bir_lowering=False, debug=False)
    x = nc.declare_dram_parameter("x", [SPC, C, N], FP32, isOutput=False)
    consts = nc.declare_dram_parameter("consts", [128, CB_COLS], FP32, isOutput=False)
    out = nc.declare_dram_parameter("out", [SPC, N], FP32, isOutput=True)
    from contextlib import ExitStack

    with tile.TileContext(nc) as tc, ExitStack() as ctx:
        _kernel_body(ctx, tc, x, consts, out)
    nc.compile()
    _NC_CACHE[key] = nc
    return nc


def const_base() -> np.ndarray:
    ct = np.zeros((128, CB_COLS), dtype=np.float32)
    # Sliding indicator band: column 7 all-ones; slice [:, 7-g:15-g] puts
    # the ones-column at position g.
    ct[:, CB_BAND + 7] = 1.0
    # Block-diagonal tridiagonal for the vertical 3-tap (both samples).
    idx = np.arange(64)
    t64 = (np.abs(idx[:, None] - idx[None, :]) <= 1).astype(np.float32)
    ct[0:64, CB_BAND2 : CB_BAND2 + 64] = t64
    ct[64:128, CB_BAND2 + 64 : CB_BAND2 + 128] = t64
    # Per-sample selectors for the softmax total + broadcast.
    ct[0:64, CB_SEL2] = 1.0
    ct[64:128, CB_SEL2 + 1] = 1.0
    ct[0, CB_SELB2 : CB_SELB2 + 64] = 1.0
    ct[1, CB_SELB2 + 64 : CB_SELB2 + 128] = 1.0
    return ct


_CT_BASE = const_base()


def make_in_maps(x: np.ndarray, prev_drop_mask: np.ndarray) -> list:
    xs = np.ascontiguousarray(np.asarray(x), dtype=np.float32).reshape(B, C, N)
    mb = (np.asarray(prev_drop_mask).astype(np.float32) * 1e30).reshape(B, H, W)
    in_maps = []
    for i in range(NCORES):
        ct = _CT_BASE.copy()
        ct[0:64, CB_MASK : CB_MASK + 64] = mb[2 * i]
        ct[64:128, CB_MASK : CB_MASK + 64] = mb[2 * i + 1]
        in_maps.append({"x": xs[i * SPC : (i + 1) * SPC], "consts": ct})
    return in_maps


def kernel(x: np.ndarray, prev_drop_mask: np.ndarray) -> np.ndarray:
    nc = _build()
    res = run_bass_kernel_spmd(nc, make_in_maps(x, prev_drop_mask), list(range(NCORES)))
    outs = [res.results[i]["out"] for i in range(NCORES)]
    return np.concatenate(outs, axis=0).reshape(B, H, W)
